# revision 3
# baseline (speedup 1.0000x reference)
"""Trainium2 Bass kernel for AnsiToPixels (embedding_lookup, memory-bound).

Computation (per glyph cell):
  raw[y,x]  = sum_ch char[ch] * glyph[ch,y,x]          (256-ch dense "one-hot" matmul)
  fg[c]     = (0.5*fg_bold+0.5) * fg_color[c]
  bg[c]     = (0.5*bg_bold+0.5) * bg_color[c]
  out[y,x,c] = raw[y,x]*(fg[c]-bg[c]) + bg[c]

Sharding: pure data parallelism over batch B=128 -> 16 per core on 8 cores,
glyph table replicated. Each core processes 25600 cells as 200 tiles of 128.

Per-tile dataflow:
  DMA in  : data[128 cells, 264] f32 (fully contiguous, 132KB)
  PE      : 2x transpose-mode [128,128] -> charT in PSUM (channel-major)
  ACT     : copy+cast PSUM->SBUF bf16
  PE      : 2x bf16 matmul (K=128 chunks) -> raw[cell, pix] in PSUM (f32)
  DVE/ACT : fused blend out[:,:,c] = raw*d_c + bg_c  (per-partition scalars)
  DMA out : per image-row segments (96B-contiguous chunks in DRAM)

d/bg are computed per group of 4 tiles from the 8 color channels in batched
DVE ops to amortize op overhead.
"""

import os
import sys

import numpy as np

for _p in ("/opt/trn_rl_repo", "/root/.axon_site/_ro/trn_rl_repo"):
    if os.path.isdir(_p) and _p not in sys.path:
        sys.path.insert(0, _p)

import concourse.bass as bass  # noqa: E402
import concourse.mybir as mybir  # noqa: E402
import concourse.tile as tile  # noqa: E402
from concourse import bacc  # noqa: E402
from concourse.bass_utils import run_bass_kernel_spmd  # noqa: E402
from concourse.masks import make_identity  # noqa: E402


def _ensure_ntff_hook():
    """Register the axon NTFF profile hook if the image's antenv lacks it,
    so run_bass_kernel_spmd(trace=True) can capture HW exec time."""
    try:
        from antenv.axon_hooks import get_axon_ntff_profile_hook  # noqa: F401

        return
    except ImportError:
        pass
    try:
        import types

        import antenv
        from trn_agent_boot.trn_boot import _ntff_profile_via_ctypes

        hook = _ntff_profile_via_ctypes("/opt/axon/libaxon_pjrt.so")
        mod = types.ModuleType("antenv.axon_hooks")
        mod.get_axon_ntff_profile_hook = lambda: hook
        mod.set_axon_ntff_profile_hook = lambda h: None
        sys.modules["antenv.axon_hooks"] = mod
        antenv.axon_hooks = mod
    except Exception as e:  # profiling is best-effort
        print(f"NTFF hook registration failed: {e}", file=sys.stderr)

N_CORES = 8
B = 128
GRID_H, GRID_W = 20, 80
GLYPH_H, GLYPH_W = 16, 8
N_GLYPHS = 256
PIX = GLYPH_H * GLYPH_W  # 128

B_SHARD = B // N_CORES  # 16
CELLS = B_SHARD * GRID_H * GRID_W  # 25600
P = 128  # cells per tile
NT = CELLS // P  # 200
GROUP = 4  # tiles per d/bg batch
ROWS = B_SHARD * GRID_H  # 320 image (b,h) rows per core
ROW_BYTES = GRID_W * GLYPH_W * 3  # 1920 floats per (row, y)

F32 = mybir.dt.float32
BF16 = mybir.dt.bfloat16


def _bcast_last(ap, n):
    """Append a stride-0 dim of size n to an AP (free-dim broadcast)."""
    return bass.AP(tensor=ap.tensor, offset=ap.offset, ap=[*ap.ap, [0, n]])


def _row_segments(t):
    """Split tile t's 128 cells into (row, w0, n, p0) segments that stay
    within one (b,h) image row (80 cells) each."""
    segs = []
    c0 = P * t
    end = P * (t + 1)
    while c0 < end:
        r, w0 = divmod(c0, GRID_W)
        n = min(GRID_W - w0, end - c0)
        segs.append((r, w0, n, c0 - P * t))
        c0 += n
    return segs


def build_kernel():
    nc = bacc.Bacc(
        "TRN2",
        target_bir_lowering=False,
        debug=False,
        enable_asserts=False,
        num_devices=N_CORES,
    )
    data = nc.dram_tensor("data", [CELLS, 264], F32, kind="ExternalInput").ap()
    glyph = nc.dram_tensor("glyph", [N_GLYPHS, PIX], F32, kind="ExternalInput").ap()
    outp = nc.dram_tensor(
        "out", [ROWS, GLYPH_H, GRID_W, GLYPH_W * 3], F32, kind="ExternalOutput"
    ).ap()

    with tile.TileContext(nc) as tc:
        with (
            tc.tile_pool(name="const", bufs=1) as const,
            tc.tile_pool(name="char", bufs=6) as char_pool,
            tc.tile_pool(name="ctbf", bufs=4) as ctbf_pool,
            tc.tile_pool(name="outsb", bufs=6) as out_pool,
            tc.tile_pool(name="grp", bufs=3) as grp_pool,
            tc.tile_pool(name="psT", bufs=3, space="PSUM") as psT,
            tc.tile_pool(name="psR", bufs=5, space="PSUM") as psR,
        ):
            ident = const.tile([128, 128], F32)
            make_identity(nc, ident[:, :])

            g32 = const.tile([128, 256], F32)
            nc.sync.dma_start(out=g32[:, 0:128], in_=glyph[0:128, :])
            nc.sync.dma_start(out=g32[:, 128:256], in_=glyph[128:256, :])
            gbf = const.tile([128, 256], BF16)
            nc.scalar.copy(gbf[:, :], g32[:, :])

            for g in range(NT // GROUP):
                tiles = range(g * GROUP, (g + 1) * GROUP)
                chars = {}
                raws = {}
                colors_g = grp_pool.tile([128, GROUP, 8], F32, tag="colors")

                for j, t in enumerate(tiles):
                    char = char_pool.tile([128, 264], F32)
                    chars[t] = char
                    nc.sync.dma_start(out=char[:, :], in_=data[t * P : (t + 1) * P, :])

                    ctps = psT.tile([128, 256], F32)
                    nc.tensor.transpose(ctps[:, 0:128], char[:, 0:128], ident[:, :])
                    nc.tensor.transpose(ctps[:, 128:256], char[:, 128:256], ident[:, :])
                    ctbf = ctbf_pool.tile([128, 256], BF16)
                    nc.scalar.copy(ctbf[:, :], ctps[:, :])

                    raw = psR.tile([128, PIX], F32)
                    raws[t] = raw
                    nc.tensor.matmul(
                        raw[:, :], ctbf[:, 0:128], gbf[:, 0:128], start=True, stop=False
                    )
                    nc.tensor.matmul(
                        raw[:, :],
                        ctbf[:, 128:256],
                        gbf[:, 128:256],
                        start=False,
                        stop=True,
                    )
                    nc.vector.tensor_copy(colors_g[:, j, :], char[:, 256:264])

                # batched d/bg for the group: channels = fgb, fgc*3, bgb, bgc*3
                sf_g = grp_pool.tile([128, GROUP], F32, tag="sf")
                sb_g = grp_pool.tile([128, GROUP], F32, tag="sb")
                fg_g = grp_pool.tile([128, GROUP, 3], F32, tag="fg")
                bg_g = grp_pool.tile([128, GROUP, 3], F32, tag="bg")
                d_g = grp_pool.tile([128, GROUP, 3], F32, tag="d")
                nc.vector.tensor_scalar(
                    out=sf_g[:, :],
                    in0=colors_g[:, :, 0],
                    scalar1=0.5,
                    scalar2=0.5,
                    op0=mybir.AluOpType.mult,
                    op1=mybir.AluOpType.add,
                )
                nc.vector.tensor_scalar(
                    out=sb_g[:, :],
                    in0=colors_g[:, :, 4],
                    scalar1=0.5,
                    scalar2=0.5,
                    op0=mybir.AluOpType.mult,
                    op1=mybir.AluOpType.add,
                )
                nc.vector.tensor_mul(
                    fg_g[:, :, :], colors_g[:, :, 1:4], _bcast_last(sf_g[:, :], 3)
                )
                nc.vector.tensor_mul(
                    bg_g[:, :, :], colors_g[:, :, 5:8], _bcast_last(sb_g[:, :], 3)
                )
                nc.vector.tensor_sub(d_g[:, :, :], fg_g[:, :, :], bg_g[:, :, :])

                for j, t in enumerate(tiles):
                    raw = raws[t]
                    out_sb = out_pool.tile([128, PIX, 3], F32)
                    # channel 0 on ACT, channels 1-2 on DVE
                    nc.scalar.activation(
                        out_sb[:, :, 0],
                        raw[:, :],
                        mybir.ActivationFunctionType.Identity,
                        bias=bg_g[:, j, 0:1],
                        scale=d_g[:, j, 0:1],
                    )
                    for c in (1, 2):
                        nc.vector.tensor_scalar(
                            out=out_sb[:, :, c],
                            in0=raw[:, :],
                            scalar1=d_g[:, j, c : c + 1],
                            scalar2=bg_g[:, j, c : c + 1],
                            op0=mybir.AluOpType.mult,
                            op1=mybir.AluOpType.add,
                        )
                    for r, w0, n, p0 in _row_segments(t):
                        nc.sync.dma_start(
                            out=outp[r, :, w0 : w0 + n, :].rearrange("y w k -> w y k"),
                            in_=out_sb[p0 : p0 + n, :, :],
                        )

    nc.compile()
    return nc


_NC = None


def _get_nc():
    global _NC
    if _NC is None:
        _NC = build_kernel()
    return _NC


def run(data, char_matrix, trace=False):
    data = np.ascontiguousarray(np.asarray(data, dtype=np.float32))
    glyph = np.ascontiguousarray(
        np.asarray(char_matrix, dtype=np.float32).reshape(N_GLYPHS, PIX)
    )
    assert data.shape == (B, GRID_H, GRID_W, 264), data.shape

    in_maps = []
    for i in range(N_CORES):
        shard = data[i * B_SHARD : (i + 1) * B_SHARD].reshape(CELLS, 264)
        in_maps.append({"data": np.ascontiguousarray(shard), "glyph": glyph})

    nc = _get_nc()
    if trace:
        _ensure_ntff_hook()
    res = run_bass_kernel_spmd(
        nc, in_maps, core_ids=list(range(N_CORES)), trace=trace
    )
    out = np.concatenate(
        [
            r["out"].reshape(B_SHARD, GRID_H * GLYPH_H, GRID_W * GLYPH_W, 3)
            for r in res.results
        ],
        axis=0,
    )
    return out, res.exec_time_ns


def kernel(data, char_matrix):
    out, _ = run(data, char_matrix, trace=False)
    return out


# revision 4
# speedup vs baseline: 1.1385x; 1.1385x over previous
"""Trainium2 Bass kernel for AnsiToPixels (embedding_lookup, memory-bound).

Computation (per glyph cell):
  raw[y,x]  = sum_ch char[ch] * glyph[ch,y,x]          (256-ch dense "one-hot" matmul)
  fg[c]     = (0.5*fg_bold+0.5) * fg_color[c]
  bg[c]     = (0.5*bg_bold+0.5) * bg_color[c]
  out[y,x,c] = raw[y,x]*(fg[c]-bg[c]) + bg[c]

Sharding: pure data parallelism over batch B=128 -> 16 per core on 8 cores,
glyph table replicated. Each core processes 25600 cells.

Layout trick for DMA efficiency: cells are assigned to partitions with an
8-way interleave ("octets"). A macro-tile covers 1024 consecutive cells;
sub-stream j holds cells {8p+j}. Partition p therefore owns 8 *consecutive*
cells of an image row, so its per-y output span is 8*8*3*4 = 768 contiguous
bytes in DRAM (8 divides the 80-cell row, so octets never straddle rows).
This keeps output DMA descriptors at 768B instead of 96B.

Per macro-tile (1024 cells = 8 sub-streams of 128):
  8x  DMA in   : data[128 cells, 264] f32 (1056B/partition, stride-8 rows)
  16x PE       : transpose-mode [128,128] -> charT in PSUM (channel-major)
  8x  ACT      : copy+cast PSUM->SBUF bf16
  16x PE       : bf16 matmul (K=128 chunks) -> raw[cell, pix] in PSUM (f32)
  5x  DVE      : batched d/bg from the 8x8 color channels
  24x DVE/ACT  : fused blend out[:, y, j, x, c] = raw*d + bg (per-partition scalars)
  ~13x DMA out : per image-row runs of partitions, 768B descriptors
"""

import os
import sys

import numpy as np

for _p in ("/opt/trn_rl_repo", "/root/.axon_site/_ro/trn_rl_repo"):
    if os.path.isdir(_p) and _p not in sys.path:
        sys.path.insert(0, _p)

import concourse.bass as bass  # noqa: E402
import concourse.mybir as mybir  # noqa: E402
import concourse.tile as tile  # noqa: E402
from concourse import bacc  # noqa: E402
from concourse.bass_utils import run_bass_kernel_spmd  # noqa: E402
from concourse.masks import make_identity  # noqa: E402


def _ensure_ntff_hook():
    """Register the axon NTFF profile hook if the image's antenv lacks it,
    so run_bass_kernel_spmd(trace=True) can capture HW exec time."""
    try:
        from antenv.axon_hooks import get_axon_ntff_profile_hook  # noqa: F401

        return
    except ImportError:
        pass
    try:
        import types

        import antenv
        from trn_agent_boot.trn_boot import _ntff_profile_via_ctypes

        hook = _ntff_profile_via_ctypes("/opt/axon/libaxon_pjrt.so")
        mod = types.ModuleType("antenv.axon_hooks")
        mod.get_axon_ntff_profile_hook = lambda: hook
        mod.set_axon_ntff_profile_hook = lambda h: None
        sys.modules["antenv.axon_hooks"] = mod
        antenv.axon_hooks = mod
    except Exception as e:  # profiling is best-effort
        print(f"NTFF hook registration failed: {e}", file=sys.stderr)


N_CORES = 8
B = 128
GRID_H, GRID_W = 20, 80
GLYPH_H, GLYPH_W = 16, 8
N_GLYPHS = 256
PIX = GLYPH_H * GLYPH_W  # 128

B_SHARD = B // N_CORES  # 16
CELLS = B_SHARD * GRID_H * GRID_W  # 25600
OCT = 8  # cells per partition (consecutive within a row)
MT = 128 * OCT  # cells per macro-tile (1024)
NT = CELLS // MT  # 25 macro-tiles
OPR = GRID_W // OCT  # octets per image row (10)
ROWS = B_SHARD * GRID_H  # 320 image (b,h) rows per core

F32 = mybir.dt.float32
BF16 = mybir.dt.bfloat16


def _bcast_last(ap, n):
    """Append a stride-0 dim of size n to an AP (free-dim broadcast)."""
    return bass.AP(tensor=ap.tensor, offset=ap.offset, ap=[*ap.ap, [0, n]])


def _oct_segments(t):
    """Split macro-tile t's 128 octets into (row, octet0_in_row, n_octets, p0)
    runs that stay within one (b,h) image row (10 octets) each."""
    segs = []
    o0 = 128 * t
    end = o0 + 128
    while o0 < end:
        r, wo = divmod(o0, OPR)
        n = min(OPR - wo, end - o0)
        segs.append((r, wo, n, o0 - 128 * t))
        o0 += n
    return segs


def build_kernel():
    nc = bacc.Bacc(
        "TRN2",
        target_bir_lowering=False,
        debug=False,
        enable_asserts=False,
        num_devices=N_CORES,
    )
    data = nc.dram_tensor("data", [CELLS, 264], F32, kind="ExternalInput").ap()
    glyph = nc.dram_tensor("glyph", [N_GLYPHS, PIX], F32, kind="ExternalInput").ap()
    outp = nc.dram_tensor(
        "out", [ROWS, GLYPH_H, GRID_W, GLYPH_W * 3], F32, kind="ExternalOutput"
    ).ap()
    # data viewed as [tile, p, j, ch]: cell = t*1024 + p*8 + j
    data_t = data.rearrange("(t p j) ch -> t p j ch", p=128, j=OCT)

    with tile.TileContext(nc) as tc:
        with (
            tc.tile_pool(name="const", bufs=1) as const,
            tc.tile_pool(name="char", bufs=12) as char_pool,
            tc.tile_pool(name="ctbf", bufs=4) as ctbf_pool,
            tc.tile_pool(name="outsb", bufs=2) as out_pool,
            tc.tile_pool(name="grp", bufs=2) as grp_pool,
            tc.tile_pool(name="psT", bufs=3, space="PSUM") as psT,
            tc.tile_pool(name="psR", bufs=5, space="PSUM") as psR,
        ):
            ident = const.tile([128, 128], F32)
            make_identity(nc, ident[:, :])

            g32 = const.tile([128, 256], F32)
            nc.sync.dma_start(out=g32[:, 0:128], in_=glyph[0:128, :])
            nc.sync.dma_start(out=g32[:, 128:256], in_=glyph[128:256, :])
            gbf = const.tile([128, 256], BF16)
            nc.scalar.copy(gbf[:, :], g32[:, :])

            for t in range(NT):
                raws = {}
                colors = grp_pool.tile([128, OCT, 8], F32, tag="colors")

                for j in range(OCT):
                    char = char_pool.tile([128, 264], F32)
                    nc.sync.dma_start(out=char[:, :], in_=data_t[t, :, j, :])

                    ctps = psT.tile([128, 256], F32)
                    nc.tensor.transpose(ctps[:, 0:128], char[:, 0:128], ident[:, :])
                    nc.tensor.transpose(ctps[:, 128:256], char[:, 128:256], ident[:, :])
                    ctbf = ctbf_pool.tile([128, 256], BF16)
                    nc.scalar.copy(ctbf[:, :], ctps[:, :])

                    raw = psR.tile([128, PIX], F32)
                    raws[j] = raw
                    nc.tensor.matmul(
                        raw[:, :], ctbf[:, 0:128], gbf[:, 0:128], start=True, stop=False
                    )
                    nc.tensor.matmul(
                        raw[:, :],
                        ctbf[:, 128:256],
                        gbf[:, 128:256],
                        start=False,
                        stop=True,
                    )
                    nc.vector.tensor_copy(colors[:, j, :], char[:, 256:264])

                # batched d/bg: color channels = fgb, fgc*3, bgb, bgc*3
                sf = grp_pool.tile([128, OCT], F32, tag="sf")
                sb = grp_pool.tile([128, OCT], F32, tag="sb")
                fg = grp_pool.tile([128, OCT, 3], F32, tag="fg")
                bg = grp_pool.tile([128, OCT, 3], F32, tag="bg")
                d = grp_pool.tile([128, OCT, 3], F32, tag="d")
                nc.vector.tensor_scalar(
                    out=sf[:, :],
                    in0=colors[:, :, 0],
                    scalar1=0.5,
                    scalar2=0.5,
                    op0=mybir.AluOpType.mult,
                    op1=mybir.AluOpType.add,
                )
                nc.vector.tensor_scalar(
                    out=sb[:, :],
                    in0=colors[:, :, 4],
                    scalar1=0.5,
                    scalar2=0.5,
                    op0=mybir.AluOpType.mult,
                    op1=mybir.AluOpType.add,
                )
                nc.vector.tensor_mul(
                    fg[:, :, :], colors[:, :, 1:4], _bcast_last(sf[:, :], 3)
                )
                nc.vector.tensor_mul(
                    bg[:, :, :], colors[:, :, 5:8], _bcast_last(sb[:, :], 3)
                )
                nc.vector.tensor_sub(d[:, :, :], fg[:, :, :], bg[:, :, :])

                # blend: out_sb[p, y, j, x, c] = raw_j[p, (y,x)] * d[p,j,c] + bg[p,j,c]
                out_sb = out_pool.tile([128, GLYPH_H, OCT, GLYPH_W, 3], F32)
                for j in range(OCT):
                    rawv = raws[j][:, :].rearrange("p (y x) -> p y x", x=GLYPH_W)
                    nc.scalar.activation(
                        out_sb[:, :, j, :, 0],
                        rawv,
                        mybir.ActivationFunctionType.Identity,
                        bias=bg[:, j, 0:1],
                        scale=d[:, j, 0:1],
                    )
                    for c in (1, 2):
                        nc.vector.tensor_scalar(
                            out=out_sb[:, :, j, :, c],
                            in0=rawv,
                            scalar1=d[:, j, c : c + 1],
                            scalar2=bg[:, j, c : c + 1],
                            op0=mybir.AluOpType.mult,
                            op1=mybir.AluOpType.add,
                        )

                for r, wo, n, p0 in _oct_segments(t):
                    nc.sync.dma_start(
                        out=outp[r, :, wo * OCT : (wo + n) * OCT, :].rearrange(
                            "y (o j) k -> o y j k", j=OCT
                        ),
                        in_=out_sb[p0 : p0 + n, :, :, :, :],
                    )

    nc.compile()
    return nc


_NC = None


def _get_nc():
    global _NC
    if _NC is None:
        _NC = build_kernel()
    return _NC


def run(data, char_matrix, trace=False):
    data = np.ascontiguousarray(np.asarray(data, dtype=np.float32))
    glyph = np.ascontiguousarray(
        np.asarray(char_matrix, dtype=np.float32).reshape(N_GLYPHS, PIX)
    )
    assert data.shape == (B, GRID_H, GRID_W, 264), data.shape

    in_maps = []
    for i in range(N_CORES):
        shard = data[i * B_SHARD : (i + 1) * B_SHARD].reshape(CELLS, 264)
        in_maps.append({"data": np.ascontiguousarray(shard), "glyph": glyph})

    nc = _get_nc()
    if trace:
        _ensure_ntff_hook()
    res = run_bass_kernel_spmd(
        nc, in_maps, core_ids=list(range(N_CORES)), trace=trace
    )
    out = np.concatenate(
        [
            r["out"].reshape(B_SHARD, GRID_H * GLYPH_H, GRID_W * GLYPH_W, 3)
            for r in res.results
        ],
        axis=0,
    )
    return out, res.exec_time_ns


def kernel(data, char_matrix):
    out, _ = run(data, char_matrix, trace=False)
    return out
